# revision 18
# baseline (speedup 1.0000x reference)
"""Trainium2 Bass kernel for a 2-layer LSTM seq2seq (CharSeq2Seq).

Data-parallel over batch: B=2048 split across 8 NeuronCores (256 rows each).
On-device layout is feature-major ("transposed"): states h/c are stored as
[128 (feature chunk), batch] tiles so the recurrent matmul needs no on-device
transposes anywhere:

    gates^T[m_gate, b] = sum_k W^T[k_feat, m_gate].T @ x_or_h^T[k_feat, b]

Matmul inputs are bf16 (fp32 PSUM accumulation); the cell state c stays fp32.
h is kept as 8 per-chunk [128, 256] tiles so a step's first matmuls depend
only on the chunks they read (produced early in the previous step), not on
the whole-h write set.

Layer-0 x-parts never touch the PE: since V=37, the entire x contribution
xg = emb[tok] @ W_ih^T + b is a 37-row table lookup, gathered on the host
(fp32, bias folded, then bf16) and shipped per step; the DVE adds it onto
the h-part PSUM in place before activation. Layer-1 keeps the x-part as
matmuls (y0 is device-resident).

Phases per core: enc L0 (spill y0 to DRAM) -> enc L1 -> dec L0 (spill y0)
-> dec L1 + fused FC per step. Cross-phase state (final h/c) is handed off
through on-chip copies, no DRAM roundtrip.
"""

import sys

sys.path.insert(0, "/opt/trn_rl_repo")

import numpy as np
import ml_dtypes

import concourse.bass as bass
import concourse.mybir as mybir
import concourse.tile as tile
from concourse import bacc
from concourse.bass_utils import run_bass_kernel_spmd

BF16 = mybir.dt.bfloat16
F32 = mybir.dt.float32

V = 37
E = 256
H = 1024
B = 2048
S = 24
T = 24
SOS = 1
NCORES = 8
BL = B // NCORES          # 256 local batch
NT = S * BL               # 6144 tokens per core
NJ = H // 128             # 8 hidden chunks
NM = 4 * H // 128         # 32 gate chunks
KX1 = H // 128            # 8 x-feature chunks for layer 1
NQ = 4                    # xg quarter-tiles per step (2 j-blocks each)
QW = NJ // NQ * 4 * BL    # cols per xg quarter

AF = mybir.ActivationFunctionType

_PROG = None  # cached compiled program


def _emit_lstm_phase(
    nc,
    pools,
    consts,
    *,
    w_dram,
    bias_key=None,  # None for L0 phases (bias folded into xg)
    kx,             # 0 for L0 (x via xg), KX1 for L1
    x_rhs=None,     # callable (t, k) -> rhs AP for x-part chunk k at step t
    xg_dram=None,   # DRAM AP [S, 128, NJ*4*BL] bf16 pre-activation x-part
    pre_step=None,  # callable (t) -> None, e.g. issue y-tile DMA load
    zero_init,      # True: h0=c0=0 (step 0 skips h-part)
    h_init=None,    # list of NJ sbuf tiles [128, BL] bf16 when not zero_init
    c_init=None,    # sbuf tile [128, NJ*BL] f32 when not zero_init
    y_out=None,     # DRAM AP [S, 128, NJ*BL] to spill per-step h
    fc=None,        # dict with fcw_sb, fcb_sb, out_dram for fused projection
):
    """One LSTM layer scan over S steps. Returns (h_chunks, c_tile)."""
    wpool = pools["w"]
    hpool = pools["h"]
    cpool = pools["c"]
    gpool = pools["g"]
    spool = pools["s"]
    lpool = pools["l"]
    pg = pools["pg"]
    bias_sb = consts[bias_key] if bias_key is not None else None

    def load_xg(t, eng=None):
        eng = eng if eng is not None else nc.sync
        qs = []
        for q in range(NQ):
            xq = pools["xg"].tile([128, QW], BF16, tag="xg", name="xg")
            eng.dma_start(xq[:], xg_dram[t][:, q * QW : (q + 1) * QW])
            qs.append(xq)
        return qs

    xg_tiles = {}
    if xg_dram is not None:
        # step-0 xg on the ACT queue: it must not sit behind the weight
        # loads on the SP queue (h(0) needs it before any weights arrive)
        xg_tiles[0] = load_xg(0, eng=nc.scalar)

    # Contraction chunks: x-part rows first (kx 128-row chunks, L1 only),
    # then the NJ hidden chunks of 128.
    wt = []
    for k in range(kx + NJ):
        w = wpool.tile([128, 4 * H], BF16, tag="w", name="w")
        nc.sync.dma_start(w[:], w_dram[k * 128 : (k + 1) * 128, :])
        wt.append(w)

    h_prev, c_prev = h_init, c_init
    for t in range(S):
        if xg_dram is not None and t + 1 < S:
            xg_tiles[t + 1] = load_xg(t + 1)  # one-step lookahead
        if pre_step is not None:
            pre_step(t)
        first = zero_init and t == 0
        h_new = []
        c_new = cpool.tile([128, NJ * BL], F32, tag="c", name="c")
        xg_t = xg_tiles.pop(t, None)
        nk_step = kx if first else kx + NJ
        for j in range(NJ):
            if xg_t is not None:
                qj = NJ // NQ
                xj = xg_t[j // qj][
                    :, (j % qj) * 4 * BL : (j % qj + 1) * 4 * BL
                ]
            if nk_step > 0:
                # One PSUM bank per gate region; k-major emission within the
                # j keeps h_k(t-1) consumers late in the in-order PE stream.
                ps = [
                    pg.tile([128, BL], F32, tag="pg", name="pg")
                    for _ in range(4)
                ]
                for ki in range(nk_step):
                    if ki < kx:
                        rhs = x_rhs(t, ki)
                    else:
                        rhs = h_prev[ki - kx][:]
                    for gi in range(4):  # i, f, g, o
                        m = gi * NJ + j
                        nc.tensor.matmul(
                            ps[gi][:],
                            wt[ki][:, m * 128 : (m + 1) * 128],
                            rhs,
                            start=(ki == 0),
                            stop=(ki == nk_step - 1),
                        )
            gs = gpool.tile([128, 4 * BL], F32, tag="g", name="g")
            if xg_t is not None:
                if first:
                    # gates = act(xg) directly; no matmul at t=0
                    nc.scalar.activation(
                        gs[:, 0 : 2 * BL], xj[:, 0 : 2 * BL], AF.Sigmoid
                    )
                    nc.scalar.activation(
                        gs[:, 2 * BL : 3 * BL], xj[:, 2 * BL : 3 * BL], AF.Tanh
                    )
                    nc.scalar.activation(
                        gs[:, 3 * BL : 4 * BL], xj[:, 3 * BL : 4 * BL], AF.Sigmoid
                    )
                else:
                    for gi, func in (
                        (0, AF.Sigmoid),
                        (1, AF.Sigmoid),
                        (2, AF.Tanh),
                        (3, AF.Sigmoid),
                    ):
                        nc.vector.tensor_add(
                            ps[gi][:], ps[gi][:],
                            xj[:, gi * BL : (gi + 1) * BL],
                        )
                        nc.scalar.activation(
                            gs[:, gi * BL : (gi + 1) * BL], ps[gi][:], func
                        )
            else:
                for gi, func in (
                    (0, AF.Sigmoid),
                    (1, AF.Sigmoid),
                    (2, AF.Tanh),
                    (3, AF.Sigmoid),
                ):
                    nc.scalar.activation(
                        gs[:, gi * BL : (gi + 1) * BL],
                        ps[gi][:],
                        func,
                        bias=bias_sb[:, gi * NJ + j : gi * NJ + j + 1],
                        scale=1.0,
                    )
            jsl = slice(j * BL, (j + 1) * BL)
            i_ap = gs[:, 0:BL]
            f_ap = gs[:, BL : 2 * BL]
            g_ap = gs[:, 2 * BL : 3 * BL]
            o_ap = gs[:, 3 * BL : 4 * BL]
            sc = spool.tile([128, BL], F32, tag="sc", name="sc")
            if first:
                nc.vector.tensor_mul(c_new[:, jsl], i_ap, g_ap)
            else:
                nc.vector.tensor_mul(sc[:], i_ap, g_ap)
                nc.vector.tensor_mul(c_new[:, jsl], f_ap, c_prev[:, jsl])
                nc.vector.tensor_add(c_new[:, jsl], c_new[:, jsl], sc[:])
            nc.scalar.activation(sc[:], c_new[:, jsl], AF.Tanh)
            hj = hpool.tile([128, BL], BF16, tag="h", name="hj")
            nc.vector.tensor_mul(hj[:], o_ap, sc[:])
            h_new.append(hj)
            if y_out is not None:
                nc.sync.dma_start(y_out[t][:, jsl], hj[:])
        if fc is not None:
            psf = pg.tile([V, BL], F32, tag="pg", name="psf")
            for k in range(NJ):
                nc.tensor.matmul(
                    psf[:],
                    fc["fcw_sb"][:, k * V : (k + 1) * V],
                    h_new[k][:],
                    start=(k == 0),
                    stop=(k == NJ - 1),
                )
            lt = lpool.tile([V, BL], F32, tag="l", name="l")
            nc.vector.tensor_scalar_add(lt[:], psf[:], fc["fcb_sb"][:])
            nc.sync.dma_start(
                fc["out_dram"][:, t * BL : (t + 1) * BL], lt[:]
            )
        h_prev, c_prev = h_new, c_new
    return h_prev, c_prev


def _save_state(nc, pools, h_chunks, c_tile):
    """Copy phase-final state to long-lived init tiles (on-chip handoff)."""
    hi = []
    for k in range(NJ):
        t = pools["h"].tile([128, BL], BF16, tag="hi", bufs=16, name="hi")
        nc.scalar.copy(t[:], h_chunks[k][:])
        hi.append(t)
    ci = pools["c"].tile([128, NJ * BL], F32, tag="ci", bufs=2, name="ci")
    nc.scalar.copy(ci[:], c_tile[:])
    return hi, ci


def _build_program(reps=1):
    """reps>1 repeats the whole pipeline in one program (timing harness)."""
    nc = bacc.Bacc("TRN2", target_bir_lowering=False, num_devices=1)

    din = lambda name, shape, dt: nc.dram_tensor(
        name, shape, dt, kind="ExternalInput"
    ).ap()
    dint = lambda name, shape, dt: nc.dram_tensor(
        name, shape, dt, kind="Internal"
    ).ap()

    xg_e0 = din("xg_e0", [S, 128, NJ * 4 * BL], BF16)
    xg_d0 = din("xg_d0", [S, 128, NJ * 4 * BL], BF16)
    w_e0 = din("w_e0", [H, 4 * H], BF16)
    w_e1 = din("w_e1", [2 * H, 4 * H], BF16)
    w_d0 = din("w_d0", [H, 4 * H], BF16)
    w_d1 = din("w_d1", [2 * H, 4 * H], BF16)
    b_e1 = din("b_e1", [128, NM], F32)
    b_d1 = din("b_d1", [128, NM], F32)
    fcw = din("fcw", [H, V], BF16)
    fcb = din("fcb", [V, 1], F32)

    y0e = dint("y0e", [S, 128, NJ * BL], BF16)
    y0d = dint("y0d", [S, 128, NJ * BL], BF16)

    logitsT = nc.dram_tensor(
        "logitsT", [V, NT], F32, kind="ExternalOutput"
    ).ap()

    with tile.TileContext(nc) as tc:
        import contextlib

        with contextlib.ExitStack() as ctx:
            pools = {
                "w": ctx.enter_context(tc.tile_pool(name="w", bufs=16)),
                "xg": ctx.enter_context(tc.tile_pool(name="xg", bufs=3)),
                "y": ctx.enter_context(tc.tile_pool(name="y", bufs=2)),
                "h": ctx.enter_context(tc.tile_pool(name="h", bufs=16)),
                "c": ctx.enter_context(tc.tile_pool(name="c", bufs=2)),
                "g": ctx.enter_context(tc.tile_pool(name="g", bufs=2)),
                # DVE is in-order, so sc single-buffering costs nothing
                "s": ctx.enter_context(tc.tile_pool(name="s", bufs=1)),
                "l": ctx.enter_context(tc.tile_pool(name="l", bufs=1)),
                "const": ctx.enter_context(tc.tile_pool(name="const", bufs=1)),
                "pg": ctx.enter_context(
                    tc.tile_pool(name="pg", bufs=8, space="PSUM")
                ),
            }
            const = pools["const"]
            consts = {}
            # consts go on the ACT engine's DMA queue so they don't delay
            # the first phase's xg/weight loads on the SP queue
            for key, drm in (("b_e1", b_e1), ("b_d1", b_d1)):
                consts[key] = const.tile([128, NM], F32, tag=key, name=key)
                nc.scalar.dma_start(consts[key][:], drm[:])
            fcw_sb = const.tile([128, NJ * V], BF16, tag="fcw", name="fcw")
            for k in range(NJ):
                nc.scalar.dma_start(
                    fcw_sb[:, k * V : (k + 1) * V],
                    fcw[k * 128 : (k + 1) * 128, :],
                )
            fcb_sb = const.tile([V, 1], F32, tag="fcb", name="fcb")
            nc.scalar.dma_start(fcb_sb[:], fcb[:])

            drams = dict(w_e0=w_e0, w_e1=w_e1, w_d0=w_d0, w_d1=w_d1)
            for _rep in range(reps):
                _emit_pipeline(
                    nc, pools, consts, fcw_sb, fcb_sb, drams,
                    xg_e0, xg_d0, y0e, y0d, logitsT,
                )

    nc.compile()
    return nc


def _emit_pipeline(
    nc, pools, consts, fcw_sb, fcb_sb, drams,
    xg_e0, xg_d0, y0e, y0d, logitsT,
):
    ycur = {}

    # ---- encoder L0 (x-part precomputed on host; h-part matmuls only) ----
    h, c = _emit_lstm_phase(
        nc, pools, consts,
        w_dram=drams["w_e0"], kx=0,
        xg_dram=xg_e0,
        zero_init=True,
        y_out=y0e,
    )
    h0i, c0i = _save_state(nc, pools, h, c)

    # ---- encoder L1 (streams y0e back per step) ----
    def pre_e1(t):
        yt = pools["y"].tile([128, NJ * BL], BF16, tag="y", name="y")
        nc.sync.dma_start(yt[:], y0e[t])
        ycur["t"] = yt

    h, c = _emit_lstm_phase(
        nc, pools, consts,
        w_dram=drams["w_e1"], bias_key="b_e1", kx=KX1,
        x_rhs=lambda t, k: ycur["t"][:, k * BL : (k + 1) * BL],
        pre_step=pre_e1,
        zero_init=True,
    )
    h1i, c1i = _save_state(nc, pools, h, c)

    # ---- decoder L0 ----
    h, c = _emit_lstm_phase(
        nc, pools, consts,
        w_dram=drams["w_d0"], kx=0,
        xg_dram=xg_d0,
        zero_init=False, h_init=h0i, c_init=c0i,
        y_out=y0d,
    )

    # ---- decoder L1 + fused FC ----
    def pre_d1(t):
        yt = pools["y"].tile([128, NJ * BL], BF16, tag="y", name="y")
        nc.sync.dma_start(yt[:], y0d[t])
        ycur["t"] = yt

    _emit_lstm_phase(
        nc, pools, consts,
        w_dram=drams["w_d1"], bias_key="b_d1", kx=KX1,
        x_rhs=lambda t, k: ycur["t"][:, k * BL : (k + 1) * BL],
        pre_step=pre_d1,
        zero_init=False, h_init=h1i, c_init=c1i,
        fc={"fcw_sb": fcw_sb, "fcb_sb": fcb_sb, "out_dram": logitsT},
    )


def _get_program():
    global _PROG
    if _PROG is None:
        _PROG = _build_program()
    return _PROG


def _prep_shared(inputs):
    emb = np.asarray(inputs["emb"], np.float32)  # [37, 256]
    shared = {}
    folds = {}
    for pre, ih, hh, bi, bh in (
        ("e0", "eW_ih0", "eW_hh0", "eb_ih0", "eb_hh0"),
        ("e1", "eW_ih1", "eW_hh1", "eb_ih1", "eb_hh1"),
        ("d0", "dW_ih0", "dW_hh0", "db_ih0", "db_hh0"),
        ("d1", "dW_ih1", "dW_hh1", "db_ih1", "db_hh1"),
    ):
        wih = np.asarray(inputs[ih], np.float32)
        whh = np.asarray(inputs[hh], np.float32)
        b = (
            np.asarray(inputs[bi], np.float32)
            + np.asarray(inputs[bh], np.float32)
        )
        if pre in ("e0", "d0"):
            # L0: x-part + bias folded into a 37-row pre-activation table,
            # reordered to the device gate layout (j-chunk, gate, partition).
            fold = emb @ wih.T + b  # [V, 4H] fp32
            # feature index = gi*H + j*128 + p  ->  cols j*4*BL + gi*BL + b
            ftab = fold.reshape(V, 4, NJ, 128).astype(ml_dtypes.bfloat16)
            folds[pre] = ftab  # [V, gi, j, p]
            shared[f"w_{pre}"] = np.ascontiguousarray(whh.T).astype(
                ml_dtypes.bfloat16
            )
        else:
            wt = np.concatenate([wih.T, whh.T], axis=0)
            shared[f"w_{pre}"] = np.ascontiguousarray(wt).astype(
                ml_dtypes.bfloat16
            )
            shared[f"b_{pre}"] = np.ascontiguousarray(b.reshape(NM, 128).T)
    shared["fcw"] = np.ascontiguousarray(
        np.asarray(inputs["fcW"], np.float32).T
    ).astype(ml_dtypes.bfloat16)
    shared["fcb"] = np.ascontiguousarray(
        np.asarray(inputs["fcb"], np.float32).reshape(V, 1)
    )
    return shared, folds


def _xg(tokens_local, ftab):
    """tokens_local [BL, S] -> xg [S, 128, NJ*4*BL] bf16 (device layout).

    ftab: [V, 4(gi), NJ(j), 128(p)] bf16 table.
    xg[t][p, j*4*BL + gi*BL + b] = ftab[tok[b, t], gi, j, p]
    """
    g = ftab[np.asarray(tokens_local)]          # [BL, S, 4, NJ, 128]
    xg = np.ascontiguousarray(g.transpose(1, 4, 3, 2, 0))  # [S,128,NJ,4,BL]
    return xg.reshape(S, 128, NJ * 4 * BL)


def build_in_maps(inputs):
    shared, folds = _prep_shared(inputs)
    src = np.asarray(inputs["src"])
    tgt = np.asarray(inputs["tgt"])
    dec = np.concatenate(
        [np.full((B, 1), SOS, dtype=tgt.dtype), tgt[:, :-1]], axis=1
    )
    in_maps = []
    for i in range(NCORES):
        sl = slice(i * BL, (i + 1) * BL)
        m = dict(shared)
        m["xg_e0"] = _xg(src[sl], folds["e0"])
        m["xg_d0"] = _xg(dec[sl], folds["d0"])
        in_maps.append(m)
    return in_maps


def kernel(**inputs):
    nc = _get_program()
    in_maps = build_in_maps(inputs)
    res = None
    for attempt in range(3):
        try:
            res = run_bass_kernel_spmd(
                nc, in_maps, core_ids=list(range(NCORES))
            )
            break
        except Exception:
            if attempt == 2:
                raise
    out = np.empty((B, T, V), np.float32)
    for i in range(NCORES):
        lt = res.results[i]["logitsT"]  # [37, T*BL]
        out[i * BL : (i + 1) * BL] = lt.reshape(V, T, BL).transpose(2, 1, 0)
    return out


if __name__ == "__main__":
    prog = _get_program()
    print("program built OK")
